# revision 1
# baseline (speedup 1.0000x reference)
"""KAN layer kernel for 8 Trainium2 NeuronCores.

Math (reference):
    basis[b,i] = sum_h silu(x[b,i]*w1[i%K,h] + b1[i%K,h]) * w2[i%K,h] + b2[i%K]
    out[b,o]   = sum_i basis[b,i] * Wsum[o,i],   Wsum = W.sum(-1)   # [O,I]

Sharding: data-parallel over the input-feature axis I (16384 -> 8 x 2048).
Each core computes a partial out[64,1024] over its feature slice; host sums.

Per-core device program (memory-bound on reading its 42 MB W slice):
  - W arrives host-transposed as Wt[i,k,o]; the k-reduction happens *inside
    the DMA* via serial accum_op=add transfers (SDMA CCE), so Wsum[i,o]
    lands in SBUF with zero engine work and contraction (i) already on the
    partition axis -- no on-chip transposes anywhere.
  - basis is computed with i on partitions: ACT evaluates
    silu(w1*x+b1) with per-partition scale/bias vectors; DVE accumulates
    w2*silu(+b2) via fused scalar_tensor_tensor. Result acc[i,b] is directly
    the matmul lhsT.
  - 2 fp32 matmuls per i-tile accumulate into PSUM across all 16 i-tiles.
"""
import numpy as np

B, I, O, K, H = 64, 16384, 1024, 5, 16
NCORES = 8
IC = I // NCORES          # 2048 features per core
P = 128                   # partition tile
NT = IC // P              # 16 i-tiles per core
NB = B                    # 64
NO = O                    # 1024
PRW = 3 * H + 1           # packed param cols per i-tile: w1,b1,w2 (16 ea) + b2
CBW = NT * NB + NT * PRW  # const tile width: x block + param block

TRACE = False             # test.py sets True to capture an NTFF profile
LAST_RESULT = None


def _build():
    from contextlib import ExitStack
    from concourse import bacc, mybir, tile

    dt = mybir.dt.float32
    nc = bacc.Bacc("TRN2", target_bir_lowering=False, debug=False,
                   num_devices=NCORES)
    Wt = nc.declare_dram_parameter("Wt", [IC, K, NO], dt, isOutput=False)
    cbd = nc.declare_dram_parameter("cb", [P, CBW], dt, isOutput=False)
    out = nc.declare_dram_parameter("out", [NB, NO], dt, isOutput=True)

    with tile.TileContext(nc) as tc, ExitStack() as ctx:
        const = ctx.enter_context(tc.tile_pool(name="const", bufs=1))
        wpool = ctx.enter_context(tc.tile_pool(name="w", bufs=8))
        bpool = ctx.enter_context(tc.tile_pool(name="basis", bufs=16))
        spool = ctx.enter_context(tc.tile_pool(name="silu", bufs=3))
        opool = ctx.enter_context(tc.tile_pool(name="out", bufs=1))
        psum = ctx.enter_context(tc.tile_pool(name="psum", bufs=1, space="PSUM"))

        cb = const.tile([P, CBW], dt)
        nc.sync.dma_start(cb[:, :], cbd[:, :])

        ps0 = psum.tile([NB, 512], dt, tag="ps0")
        ps1 = psum.tile([NB, 512], dt, tag="ps1")

        # ---- basisT[i,b] for every i-tile (ACT/DVE only; no W dependency) ----
        accs = []
        for t in range(NT):
            xs = cb[:, t * NB:(t + 1) * NB]
            pb = NT * NB + t * PRW
            acc = bpool.tile([P, NB], dt)
            for h in range(H):
                st = spool.tile([P, NB], dt)
                nc.scalar.activation(
                    st[:, :], xs, mybir.ActivationFunctionType.Silu,
                    bias=cb[:, pb + H + h:pb + H + h + 1],
                    scale=cb[:, pb + h:pb + h + 1],
                )
                if h == 0:
                    # acc = w2[:,0]*silu + b2
                    nc.vector.tensor_scalar(
                        acc[:, :], st[:, :],
                        cb[:, pb + 2 * H:pb + 2 * H + 1],
                        cb[:, pb + 3 * H:pb + 3 * H + 1],
                        op0=mybir.AluOpType.mult, op1=mybir.AluOpType.add,
                    )
                else:
                    # acc = w2[:,h]*silu + acc
                    nc.vector.scalar_tensor_tensor(
                        acc[:, :], st[:, :],
                        cb[:, pb + 2 * H + h:pb + 2 * H + h + 1],
                        acc[:, :],
                        op0=mybir.AluOpType.mult, op1=mybir.AluOpType.add,
                    )
            accs.append(acc)

        # ---- Wsum[i,o] = sum_k Wt[i,k,o], reduced inside the DMA.
        # All SWDGE DMAs issue in program order from the one gpsimd
        # sequencer, and step k of a tile must wait for step k-1's
        # completion (~2us). Interleaving the chains of a window of tiles
        # keeps every wait pre-satisfied so the queue never stalls. ----
        WIN = 4
        wsums = [None] * NT
        for base in range(0, NT, WIN):
            grp = range(base, min(base + WIN, NT))
            for t in grp:
                wsums[t] = wpool.tile([P, NO], dt, tag="wsum", name=f"wsum{t}")
            for k in range(K):
                for t in grp:
                    nc.gpsimd.dma_start(
                        wsums[t][:, :], Wt[t * P:(t + 1) * P, k, :],
                        accum_op=(mybir.AluOpType.bypass if k == 0
                                  else mybir.AluOpType.add))

        # ---- partial matmuls: out[b,o] += basisT.T @ Wsum ----
        for t in range(NT):
            nc.tensor.matmul(ps0[:, :], accs[t][:, :], wsums[t][:, 0:512],
                             start=(t == 0), stop=(t == NT - 1))
            nc.tensor.matmul(ps1[:, :], accs[t][:, :], wsums[t][:, 512:1024],
                             start=(t == 0), stop=(t == NT - 1))

        out_sb = opool.tile([NB, NO], dt)
        nc.vector.tensor_copy(out_sb[:, 0:512], ps0[:, :])
        nc.vector.tensor_copy(out_sb[:, 512:1024], ps1[:, :])
        nc.sync.dma_start(out[:, :], out_sb[:, :])
    nc.compile()
    return nc


def kernel(x, w1, b1, w2, b2, W):
    global LAST_RESULT
    from concourse.bass_utils import run_bass_kernel_spmd

    x = np.asarray(x, dtype=np.float32)
    W = np.asarray(W, dtype=np.float32)
    w1 = np.asarray(w1, dtype=np.float32)
    b1 = np.asarray(b1, dtype=np.float32)
    w2 = np.asarray(w2, dtype=np.float32)
    b2 = np.asarray(b2, dtype=np.float32)

    # ---- host prep: W -> [I,K,O] (contraction-major layout for the PE) ----
    Wt_full = np.ascontiguousarray(W.reshape(O, I * K).T).reshape(I, K, O)

    idx = np.arange(I) % K
    w1e, b1e, w2e = w1[idx], b1[idx], w2[idx]          # [I,H]
    b2e = b2[idx][:, None]                             # [I,1]
    pr = np.concatenate([w1e, b1e, w2e, b2e], axis=1)  # [I, PRW]

    in_maps = []
    for c in range(NCORES):
        sl = slice(c * IC, (c + 1) * IC)
        # x slice, transposed to [i, b], then swizzled to SBUF layout [P, NT*NB]
        xt = np.ascontiguousarray(x[:, sl].T)          # [IC, NB]
        xt_sb = xt.reshape(NT, P, NB).transpose(1, 0, 2).reshape(P, NT * NB)
        pr_sb = pr[sl].reshape(NT, P, PRW).transpose(1, 0, 2).reshape(P, NT * PRW)
        cb = np.ascontiguousarray(
            np.concatenate([xt_sb, pr_sb], axis=1), dtype=np.float32)
        in_maps.append({"Wt": Wt_full[sl], "cb": cb})

    nc = _build()
    res = run_bass_kernel_spmd(nc, in_maps, list(range(NCORES)), trace=TRACE)
    LAST_RESULT = res
    out = np.zeros((B, O), dtype=np.float32)
    for c in range(NCORES):
        out += res.results[c]["out"]
    return out



# revision 4
# speedup vs baseline: 2.5422x; 2.5422x over previous
"""KAN layer kernel for 8 Trainium2 NeuronCores.

Math (reference):
    basis[b,i] = sum_h silu(x[b,i]*w1[i%K,h] + b1[i%K,h]) * w2[i%K,h] + b2[i%K]
    out[b,o]   = sum_i basis[b,i] * Wsum[o,i],   Wsum = W.sum(-1)   # [O,I]

Sharding: data-parallel over the input-feature axis I (16384 -> 8 x 2048).
Each core computes a partial out[64,1024] over its feature slice; host sums.

Per-core device program (memory-bound on reading its W slice):
  - W is host-cast to bf16 (tolerance 2e-2 >> bf16 error ~3e-3), halving
    HBM traffic vs fp32. Layout Wt[i, (k,o)]: 16 plain HWDGE loads of
    [128, 5120] on the Sync queue -- no DMA-accum, no SWDGE.
  - The k-reduction rides the PE: out[b,o] = sum_{i,k} basis[b,i]*W[o,i,k],
    so each i-tile issues 5 (k) x 2 (O-half) matmuls reusing the same
    lhsT = basisT tile, accumulating all 80 into each PSUM bank.
  - basis is computed with i on partitions in 6 wide ops per i-tile
    (vs 32 narrow ones): two DVE broadcast tensor_tensor (x*w1+b1 over
    [128,64,16]), one big ACT silu, one DVE w2 mult, one DVE h-reduce,
    one DVE +b2/cast-to-bf16.
"""
import numpy as np

B, I, O, K, H = 64, 16384, 1024, 5, 16
NCORES = 8
IC = I // NCORES          # 2048 features per core
P = 128                   # partition tile
NT = IC // P              # 16 i-tiles per core
NB = B                    # 64
NO = O                    # 1024
ROW = K * NO              # 5120 bf16 per Wt row
# cb16 (bf16) column blocks: xs [NT*NB] | w1 [NT*H] | b1 [NT*H] | w2 [NT*H]
X0 = NT * NB
X1 = X0 + NT * H
X2 = X1 + NT * H
CBW = X2 + NT * H

TRACE = False             # test.py sets True to capture an NTFF profile
LAST_RESULT = None


def _build():
    from contextlib import ExitStack
    from concourse import bacc, mybir, tile

    f32 = mybir.dt.float32
    bf16 = mybir.dt.bfloat16
    nc = bacc.Bacc("TRN2", target_bir_lowering=False, debug=False,
                   num_devices=NCORES)
    Wt = nc.declare_dram_parameter("Wt", [IC, ROW], bf16, isOutput=False)
    cb16d = nc.declare_dram_parameter("cb16", [P, CBW], bf16, isOutput=False)
    cb32d = nc.declare_dram_parameter("cb32", [P, NT], f32, isOutput=False)
    out = nc.declare_dram_parameter("out", [NB, NO], f32, isOutput=True)

    with tile.TileContext(nc) as tc, ExitStack() as ctx:
        const = ctx.enter_context(tc.tile_pool(name="const", bufs=1))
        wpool = ctx.enter_context(tc.tile_pool(name="w", bufs=4))
        ppool = ctx.enter_context(tc.tile_pool(name="pre", bufs=3))
        spool = ctx.enter_context(tc.tile_pool(name="silu", bufs=2))
        mpool = ctx.enter_context(tc.tile_pool(name="msum", bufs=2))
        apool = ctx.enter_context(tc.tile_pool(name="acc", bufs=NT))
        opool = ctx.enter_context(tc.tile_pool(name="out", bufs=1))
        psum = ctx.enter_context(tc.tile_pool(name="psum", bufs=1, space="PSUM"))

        cb16 = const.tile([P, CBW], bf16)
        cb32 = const.tile([P, NT], f32)
        nc.scalar.dma_start(cb16[:, :], cb16d[:, :])
        nc.scalar.dma_start(cb32[:, :], cb32d[:, :])

        ps0 = psum.tile([NB, 512], f32, tag="ps0")
        ps1 = psum.tile([NB, 512], f32, tag="ps1")

        # ---- W tile loads: 16 plain bf16 DMAs on the Sync HWDGE queue ----
        wts = []
        for t in range(NT):
            wt = wpool.tile([P, ROW], bf16, tag="wt")
            nc.sync.dma_start(wt[:, :], Wt[t * P:(t + 1) * P, :])
            wts.append(wt)

        # ---- basisT[i,b] per i-tile (DVE/ACT only; no W dependency) ----
        accs = []
        for t in range(NT):
            xs = cb16[:, t * NB:(t + 1) * NB]              # [P, 64]
            w1s = cb16[:, X0 + t * H:X0 + (t + 1) * H]     # [P, 16]
            b1s = cb16[:, X1 + t * H:X1 + (t + 1) * H]
            w2s = cb16[:, X2 + t * H:X2 + (t + 1) * H]
            pre = ppool.tile([P, NB, H], bf16)
            nc.vector.tensor_tensor(
                pre[:, :, :],
                xs[:, :, None].to_broadcast([P, NB, H]),
                w1s[:, None, :].to_broadcast([P, NB, H]),
                mybir.AluOpType.mult)
            pre2 = ppool.tile([P, NB, H], bf16)
            nc.vector.tensor_tensor(
                pre2[:, :, :], pre[:, :, :],
                b1s[:, None, :].to_broadcast([P, NB, H]),
                mybir.AluOpType.add)
            s = spool.tile([P, NB, H], bf16)
            nc.scalar.activation(s[:, :, :], pre2[:, :, :],
                                 mybir.ActivationFunctionType.Silu)
            sw = spool.tile([P, NB, H], bf16)
            nc.vector.tensor_tensor(
                sw[:, :, :], s[:, :, :],
                w2s[:, None, :].to_broadcast([P, NB, H]),
                mybir.AluOpType.mult)
            bsum = mpool.tile([P, NB], f32)
            nc.vector.tensor_reduce(bsum[:, :], sw[:, :, :],
                                    axis=mybir.AxisListType.X,
                                    op=mybir.AluOpType.add)
            acc = apool.tile([P, NB], bf16, tag="acc")
            nc.vector.tensor_scalar_add(acc[:, :], bsum[:, :],
                                        cb32[:, t:t + 1])
            accs.append(acc)

        # ---- partial matmuls: out[b,o] += sum_k basisT.T @ W[:,k,:] ----
        for t in range(NT):
            for k in range(K):
                first = (t == 0 and k == 0)
                last = (t == NT - 1 and k == K - 1)
                nc.tensor.matmul(ps0[:, :], accs[t][:, :],
                                 wts[t][:, k * NO:k * NO + 512],
                                 start=first, stop=last)
                nc.tensor.matmul(ps1[:, :], accs[t][:, :],
                                 wts[t][:, k * NO + 512:(k + 1) * NO],
                                 start=first, stop=last)

        out_sb = opool.tile([NB, NO], f32)
        nc.vector.tensor_copy(out_sb[:, 0:512], ps0[:, :])
        nc.vector.tensor_copy(out_sb[:, 512:1024], ps1[:, :])
        nc.sync.dma_start(out[:, :], out_sb[:, :])
    nc.compile()
    return nc


def kernel(x, w1, b1, w2, b2, W):
    global LAST_RESULT
    import ml_dtypes
    from concourse.bass_utils import run_bass_kernel_spmd

    bf16 = ml_dtypes.bfloat16
    x = np.asarray(x, dtype=np.float32)
    W = np.asarray(W, dtype=np.float32)
    w1 = np.asarray(w1, dtype=np.float32)
    b1 = np.asarray(b1, dtype=np.float32)
    w2 = np.asarray(w2, dtype=np.float32)
    b2 = np.asarray(b2, dtype=np.float32)

    # ---- host prep: W -> bf16 [I, K*O] (i-major rows, k-major in-row) ----
    Wb = W.astype(bf16).view(np.uint16)                # [O, I, K]
    Wt_full = np.ascontiguousarray(Wb.transpose(1, 2, 0))  # [I, K, O] u16
    Wt_full = Wt_full.reshape(I, ROW).view(bf16)

    idx = np.arange(I) % K
    w1e = w1[idx].astype(bf16)                         # [I, H]
    b1e = b1[idx].astype(bf16)
    w2e = w2[idx].astype(bf16)
    b2e = b2[idx].astype(np.float32)                   # [I]

    def swz(a, cols):
        # [IC, cols] -> SBUF layout [P, NT*cols] (tile-major along free dim)
        return np.ascontiguousarray(
            a.reshape(NT, P, cols).transpose(1, 0, 2).reshape(P, NT * cols))

    x_bf = x.astype(bf16)
    in_maps = []
    for c in range(NCORES):
        sl = slice(c * IC, (c + 1) * IC)
        xt = np.ascontiguousarray(x_bf[:, sl].T)       # [IC, NB] bf16
        cb16 = np.concatenate(
            [swz(xt, NB), swz(w1e[sl], H), swz(b1e[sl], H), swz(w2e[sl], H)],
            axis=1)
        cb32 = swz(b2e[sl][:, None], 1)                # [P, NT] f32
        in_maps.append({
            "Wt": np.ascontiguousarray(Wt_full[sl]),
            "cb16": np.ascontiguousarray(cb16),
            "cb32": np.ascontiguousarray(cb32),
        })

    nc = _build()
    res = run_bass_kernel_spmd(nc, in_maps, list(range(NCORES)), trace=TRACE)
    LAST_RESULT = res
    out = np.zeros((B, O), dtype=np.float32)
    for c in range(NCORES):
        out += res.results[c]["out"]
    return out
